# revision 12
# baseline (speedup 1.0000x reference)
"""Trainium2 Bass kernel for nn_FFTDiffuseBase.

Self-contained: builds constants on host, emits a Tile/Bass SPMD program for
8 NeuronCores (2 images x 4-way column split with ghost-halo exchange),
runs via bass_utils.run_bass_kernel_spmd, gathers the full output.
"""
import numpy as np
from contextlib import ExitStack

import concourse.bass as bass
import concourse.tile as tile
from concourse import bacc
from concourse import mybir
from concourse.bass_utils import run_bass_kernel_spmd

F32 = mybir.dt.float32
F16 = mybir.dt.float16
ALU = mybir.AluOpType
ACTF = mybir.ActivationFunctionType

BS = 128
OVERLAP = 16
NITER = 250
L = 0.24
FFT_STEPS = 10
DEPS = 0.1
H = 512
W = 512

K = np.exp(np.log(np.float32(0.03), dtype=np.float32), dtype=np.float32)

YS = list(range(0, H, BS - OVERLAP))          # [0,112,224,336,448]
XS = list(range(0, W, BS - OVERLAP))
YE = [min(y + BS, H) for y in YS]
XE = [min(x + BS, W) for x in XS]


# ---------------------------------------------------------------- host consts
def dft_mats(h, w):
    w2 = w // 2 + 1
    n = np.arange(w)
    kw = np.arange(w2)
    ang_w = 2 * np.pi * np.outer(kw, n) / w
    CwT = np.cos(ang_w).T.astype(np.float32)
    SwT = (-np.sin(ang_w)).T.astype(np.float32)
    m = np.arange(h)
    ang_h = 2 * np.pi * np.outer(m, m) / h
    Ch = np.cos(ang_h).astype(np.float32)
    Sh = np.sin(ang_h).astype(np.float32)
    alpha = np.full(w2, 2.0)
    alpha[0] = 1.0
    if w % 2 == 0:
        alpha[-1] = 1.0
    A = (alpha[:, None] * np.cos(ang_w) / (h * w)).astype(np.float32)
    Bm = (-alpha[:, None] * np.sin(ang_w) / (h * w)).astype(np.float32)
    return CwT, SwT, Ch, Sh, A, Bm


_CONSTS_CACHE = {}


def make_consts():
    if "c" in _CONSTS_CACHE:
        return _CONSTS_CACHE["c"]
    c = {}
    ii = np.eye(128, dtype=np.float32)
    c["id128"] = ii
    c["negid128"] = -ii
    sup = np.zeros((128, 128), np.float32)   # out[m] = in[m+1]
    for m_ in range(127):
        sup[m_ + 1, m_] = 1.0
    c["sup128"] = sup
    sdn = np.zeros((128, 128), np.float32)   # out[m] = in[m-1]
    for m_ in range(1, 128):
        sdn[m_ - 1, m_] = 1.0
    c["sdn128"] = sdn
    c["sdnneg128"] = -sdn
    c["id16"] = ii.astype(np.float16)
    c["negid16"] = (-ii).astype(np.float16)
    c["sup16"] = sup.astype(np.float16)
    c["sdnneg16"] = (-sdn).astype(np.float16)
    for h in (128, 64):
        _, _, Ch, Sh, _, _ = dft_mats(h, h)
        c[f"chm{h}"] = Ch.astype(np.float16)
        c[f"shm{h}"] = Sh.astype(np.float16)
        c[f"shneg{h}"] = (-Sh).astype(np.float16)
        ky = np.arange(h).astype(np.float32) * np.float32(2 * np.pi / h)
        ky1c = (np.float32(1.0) - np.cos(ky)).astype(np.float32)
        c[f"kymat_{h}"] = np.tile(ky1c[:, None], (1, 65)).copy()       # [h,65]
    for w in (128, 64):
        w2 = w // 2 + 1
        CwT, SwT, _, _, _, _ = dft_mats(128, w)
        c[f"cwt{w}"] = CwT.astype(np.float16)
        c[f"swt{w}"] = SwT.astype(np.float16)
        kx = np.arange(w2).astype(np.float32) * np.float32(2 * np.pi / w)
        kx1c = (np.float32(1.0) - np.cos(kx)).astype(np.float32)
        c[f"kxmat_{w}"] = np.tile(kx1c[None, :], (128, 1)).copy()      # [128,w2]
    for h in (128, 64):
        for w in (128, 64):
            _, _, _, _, A, Bm = dft_mats(h, w)
            c[f"amat_{h}_{w}"] = A.astype(np.float16)
            c[f"bmat_{h}_{w}"] = Bm.astype(np.float16)
    bx = np.where(np.arange(128) < OVERLAP, np.arange(128) / OVERLAP, 1.0)
    c["bx128"] = np.tile(bx.astype(np.float32)[None, :], (128, 1)).copy()
    c["by128"] = bx.astype(np.float32)[:, None].copy()                 # [128,1]
    bmfull = np.zeros((128, 5), np.float32)
    bmcorrneg = np.zeros((128, 5), np.float32)
    for yr in range(5):
        bmfull[YS[yr] // 4: YE[yr] // 4, yr] = 1.0
        if YE[yr] < H:
            bmcorrneg[YE[yr] // 4 - 1, yr] = -1.0
    c["bmfull"] = bmfull
    c["bmcorrneg"] = bmcorrneg
    selyr = np.zeros((5, 5 * 128), np.float32)
    for yr in range(5):
        selyr[yr, yr * 128:(yr + 1) * 128] = 1.0
    c["selyr"] = selyr
    for q in ("cv", "ch", "ur"):
        cnt = np.zeros((5, 5), np.float32)
        for yr in range(5):
            ye = {"cv": min(YE[yr] - 1, H - 1), "ch": YE[yr],
                  "ur": min(YE[yr], H - 1)}[q]
            for xr in range(5):
                xe = {"cv": min(XE[xr], W), "ch": min(XE[xr] - 1, W - 1),
                      "ur": min(XE[xr], W - 1)}[q]
                cnt[yr, xr] = 1.0 / ((ye - YS[yr]) * (xe - XS[xr]))
        c[f"cnt_{q}"] = cnt
    c["ones1x128"] = np.ones((1, 128), np.float32)
    _CONSTS_CACHE["c"] = c
    return c


def make_core_consts(g, nsplit):
    sel = np.zeros((128, nsplit), np.float32)
    sel[:, g] = 1.0
    selprev = np.zeros((128, nsplit), np.float32)
    if g > 0:
        selprev[:, g - 1] = 1.0
    selnext = np.zeros((128, nsplit), np.float32)
    if g < nsplit - 1:
        selnext[:, g + 1] = 1.0
    return {"sel": sel, "selprev": selprev, "selnext": selnext}


def r3(t, j=4):
    """[128, (j w)] AP -> [128, j, w]"""
    return t[:].rearrange("p (j w) -> p j w", j=j)


# ---------------------------------------------------------------- program
def build_program(nsplit=4, G=16, niter=NITER, nblocks=None,
                  ndum=6, ndum_exch=20):
    assert 512 % nsplit == 0
    OWN = 512 // nsplit
    if nsplit == 1:
        G = 0
    Wc = OWN + 2 * G
    W4 = 4 * Wc
    P0 = max(G, 1)
    Wp2 = 512 + 2 * P0
    Wp = 512 + 2 * G

    nc = bacc.Bacc("TRN2", target_bir_lowering=False, num_devices=8)

    guide = nc.dram_tensor("guide", [3, H, W], F32, kind="ExternalInput")
    initial = nc.dram_tensor("initial", [H, W], F32, kind="ExternalInput")
    out = nc.dram_tensor("out", [H, W], F32, kind="ExternalOutput")

    consts = make_consts()

    def _dt(v):
        if v.dtype == np.float32:
            return F32
        if v.dtype == np.float16:
            return F16
        return mybir.dt.bfloat16
    cdram = {k: nc.dram_tensor(k, list(v.shape), _dt(v), kind="ExternalInput")
             for k, v in consts.items()}
    for k in ("sel", "selprev", "selnext"):
        cdram[k] = nc.dram_tensor(k, [128, nsplit], F32, kind="ExternalInput")

    blocks = [(yi, xi) for yi in range(5) for xi in range(5)]
    blocks.sort(key=lambda b: (2 * b[0] + b[1], b[0]))   # wavefront order
    if nblocks is not None:
        blocks = blocks[:nblocks]

    with tile.TileContext(nc) as tc, ExitStack() as ctx:
        cp = ctx.enter_context(tc.tile_pool(name="consts", bufs=1))
        C = {}
        for k, dh_ in cdram.items():
            t = cp.tile(list(dh_.shape), dh_.dtype, tag=k)
            nc.sync.dma_start(t[:], dh_[:])
            C[k] = t

        dram = ctx.enter_context(tc.tile_pool(name="dram", bufs=1, space="DRAM"))
        depth_pad = dram.tile([H, Wp], F16)

        coef = ctx.enter_context(tc.tile_pool(name="coef", bufs=1))
        cv = coef.tile([128, 4, Wp2], F32, tag="cv", name="cv")
        ch = coef.tile([128, 4, Wp2], F32, tag="ch", name="ch")
        urf = coef.tile([128, 4, 512], F32, tag="urf", name="urf")
        meansb = coef.tile([128, 75], F32, tag="meansb", name="meansb")
        gateb = coef.tile([128, 25], F32, tag="gateb", name="gateb")
        shiftb = coef.tile([128, 1], F32, tag="shiftb", name="shiftb")
        zrow = coef.tile([1, 512], F32, tag="zrow", name="zrow")
        nc.vector.memset(zrow[:], 0.0)

        # ============================== phase A: cv/ch from guide ===========
        with tc.tile_pool(name="pa", bufs=1) as pa, \
             tc.tile_pool(name="pa_ps", bufs=3, space="PSUM") as paps:
            nc.vector.memset(cv[:], 0.0)
            nc.gpsimd.memset(ch[:], 0.0)
            accv = pa.tile([128, 2048], F32, tag="accv", name="accv")
            acch = pa.tile([128, 2048], F32, tag="acch", name="acch")
            nc.gpsimd.memset(acch[:], 0.0)
            gb = [pa.tile([128, 2048], F32, tag=f"gb{c_}", name=f"gb{c_}") for c_ in range(3)]
            for c_ in range(3):
                nc.sync.dma_start(
                    gb[c_][:],
                    guide[c_].rearrange("(p j) w -> p (j w)", p=128))
            for c_ in range(3):
                fr = paps.tile([128, 512], F32, tag="fr", name="fr")
                nc.tensor.matmul(fr[:], C["sup128"][:], gb[c_][:, 0:512],
                                 start=True, stop=True)
                tmp = pa.tile([128, 2048], F32, tag="tmpv", name="tmpv")
                nc.vector.tensor_sub(tmp[:, 0:1536], gb[c_][:, 512:2048],
                                     gb[c_][:, 0:1536])
                nc.vector.tensor_sub(tmp[:, 1536:2048], fr[:],
                                     gb[c_][:, 1536:2048])
                if c_ == 0:
                    nc.vector.scalar_tensor_tensor(
                        accv[:], tmp[:], -1.0, tmp[:],
                        op0=ALU.mult, op1=ALU.max)
                else:
                    ab = pa.tile([128, 2048], F32, tag="absv", name="absv")
                    nc.vector.scalar_tensor_tensor(
                        ab[:], tmp[:], -1.0, tmp[:], op0=ALU.mult, op1=ALU.max)
                    nc.vector.tensor_add(accv[:], accv[:], ab[:])
                tmph = pa.tile([128, 2048], F32, tag="tmph", name="tmph")
                nc.gpsimd.tensor_sub(tmph[:, 0:2047], gb[c_][:, 1:2048],
                                     gb[c_][:, 0:2047])
                if c_ == 0:
                    nc.vector.scalar_tensor_tensor(
                        acch[:, 0:2047], tmph[:, 0:2047], -1.0,
                        tmph[:, 0:2047], op0=ALU.mult, op1=ALU.max)
                else:
                    ab2 = pa.tile([128, 2048], F32, tag="absh", name="absh")
                    nc.vector.scalar_tensor_tensor(
                        ab2[:, 0:2047], tmph[:, 0:2047], -1.0,
                        tmph[:, 0:2047], op0=ALU.mult, op1=ALU.max)
                    nc.gpsimd.tensor_add(acch[:, 0:2047], acch[:, 0:2047],
                                         ab2[:, 0:2047])
            s13k = float(1.0 / (3.0 * float(K)))
            for acc, dst in ((accv, cv), (acch, ch)):
                t1 = pa.tile([128, 2048], F32, tag="gt1", name="gt1")
                nc.scalar.activation(t1[:], acc[:], ACTF.Square, scale=s13k)
                nc.vector.tensor_scalar_add(t1[:], t1[:], 1.0)
                nc.vector.reciprocal_approx_fast(dst[:, :, P0:P0 + 512],
                                                 r3(t1))
            nc.sync.dma_start(cv[127:128, 3, P0:P0 + 512],
                              zrow[0:1, 0:512])                  # row 511
            nc.vector.memset(ch[:, :, P0 + 511], 0.0)            # col 511

        # ============================== phase B: ur =========================
        with tc.tile_pool(name="pb", bufs=1) as pb, \
             tc.tile_pool(name="pb_ps", bufs=4, space="PSUM") as pbps:
            varr = {}
            for side, src in (("cv", cv), ("ch", ch)):
                sq = pb.tile([128, 4, Wp2], F32, tag="sq", name="sq")
                nc.vector.memset(sq[:], 0.0)
                nc.scalar.activation(sq[:, :, P0:P0 + 512],
                                     src[:, :, P0:P0 + 512], ACTF.Square)
                for which, xt in (("s1", src), ("s2", sq)):
                    hs = pb.tile([128, 4, 512], F32, tag=f"hs{which}", name=f"hs{which}")
                    a0 = xt[:, :, P0 - 1:P0 - 1 + 512]
                    a1 = xt[:, :, P0:P0 + 512]
                    a2 = xt[:, :, P0 + 1:P0 + 1 + 512]
                    nc.vector.tensor_add(hs[:], a0, a1)
                    nc.vector.tensor_add(hs[:], hs[:], a2)
                    hsf = hs[:].rearrange("p j w -> p (j w)")
                    vs = pb.tile([128, 2048], F32, tag=f"vs{which}", name=f"vs{which}")
                    nc.vector.tensor_add(vs[:, 512:1536], hsf[:, 0:1024],
                                         hsf[:, 512:1536])
                    nc.vector.tensor_add(vs[:, 512:1536], vs[:, 512:1536],
                                         hsf[:, 1024:2048])
                    pv = pbps.tile([128, 512], F32, tag="pv", name="pv")
                    nc.tensor.matmul(pv[:], C["sdn128"][:], hsf[:, 1536:2048],
                                     start=True, stop=True)
                    nv = pbps.tile([128, 512], F32, tag="nv", name="nv")
                    nc.tensor.matmul(nv[:], C["sup128"][:], hsf[:, 0:512],
                                     start=True, stop=True)
                    nc.vector.tensor_add(vs[:, 0:512], hsf[:, 0:512],
                                         hsf[:, 512:1024])
                    nc.vector.tensor_add(vs[:, 0:512], vs[:, 0:512], pv[:])
                    nc.vector.tensor_add(vs[:, 1536:2048], hsf[:, 1024:1536],
                                         hsf[:, 1536:2048])
                    nc.vector.tensor_add(vs[:, 1536:2048], vs[:, 1536:2048],
                                         nv[:])
                    varr[which] = vs
                vv = pb.tile([128, 2048], F32, tag=f"vv{side}", name=f"vv{side}")
                nc.vector.tensor_scalar_mul(vv[:], varr["s2"][:],
                                            float(1.0 / 9.0))
                m1 = pb.tile([128, 2048], F32, tag="m1", name="m1")
                nc.scalar.activation(m1[:], varr["s1"][:], ACTF.Square,
                                     scale=float(1.0 / 9.0))
                nc.vector.tensor_sub(vv[:], vv[:], m1[:])
                varr[side] = vv
            ua = pb.tile([128, 2048], F32, tag="ua", name="ua")
            nc.vector.tensor_scalar(ua[:], varr["cv"][:], 0.1, None,
                                    op0=ALU.is_lt)
            nc.vector.scalar_tensor_tensor(
                urf[:].rearrange("p j w -> p (j w)"), varr["ch"][:], 0.1,
                ua[:], op0=ALU.is_lt, op1=ALU.mult)
            nc.sync.dma_start(urf[127:128, 3, :], zrow[0:1, 0:512])  # row 511

        # ============================== phase C: block means ================
        with tc.tile_pool(name="pc", bufs=1) as pc, \
             tc.tile_pool(name="pc_ps", bufs=1, space="PSUM") as pcps:
            sums = {}
            for q, src, corr in (
                    ("cv", cv[:, :, P0:P0 + 512], True),
                    ("ch", ch[:, :, P0:P0 + 512], False),
                    ("ur", urf[:, :, :], False)):
                rg = pcps.tile([128, 4, 512], F32, tag="rg", name="rg")
                for jj in range(4):
                    nc.tensor.matmul(rg[0:5, jj, :], C["bmfull"][:],
                                     src[:, jj, :], start=True,
                                     stop=(jj < 3 or not corr))
                if corr:
                    nc.tensor.matmul(rg[0:5, 3, :], C["bmcorrneg"][:],
                                     src[:, 3, :], start=False, stop=True)
                sm = pc.tile([5, 5], F32, tag=f"sm{q}", name=f"sm{q}")
                for xr in range(5):
                    x0 = XS[xr]
                    x1 = {"cv": min(XE[xr], W), "ch": min(XE[xr] - 1, W - 1),
                          "ur": min(XE[xr], W - 1)}[q]
                    nc.vector.tensor_reduce(
                        sm[0:5, xr:xr + 1], rg[0:5, :, x0:x1],
                        axis=mybir.AxisListType.XY, op=ALU.add)
                nc.vector.tensor_mul(sm[:], sm[:], C[f"cnt_{q}"][:])
                sums[q] = sm
            mscr = dram.tile([75], F32, name="mscr")
            mscr3 = mscr[:].rearrange("(q a b) -> q a b", a=5, b=5)
            for qi, q in enumerate(("cv", "ch", "ur")):
                nc.sync.dma_start(mscr3[qi], sums[q][:])
            nc.sync.dma_start(meansb[:], mscr[:].partition_broadcast(128))
            nc.vector.tensor_scalar(gateb[:], meansb[:, 50:75], 0.7, None,
                                    op0=ALU.is_gt)

        # ============================== phase D: shift + depth init =========
        dp3 = depth_pad[:].rearrange("(p j) w -> p j w", p=128)
        with tc.tile_pool(name="pd", bufs=1) as pd, \
             tc.tile_pool(name="pd_ps", bufs=1, space="PSUM") as pdps:
            iband = pd.tile([128, 2048], F32, tag="iband", name="iband")
            nc.sync.dma_start(iband[:],
                              initial[:].rearrange("(p j) w -> p (j w)", p=128))
            mn = pd.tile([128, 1], F32, tag="mn", name="mn")
            nc.vector.tensor_reduce(mn[:], iband[:],
                                    axis=mybir.AxisListType.X, op=ALU.min)
            nc.vector.tensor_scalar_mul(mn[:], mn[:], -1.0)
            mn1 = pd.tile([1, 1], F32, tag="mn1", name="mn1")
            nc.gpsimd.tensor_reduce(mn1[:], mn[:],
                                    axis=mybir.AxisListType.C, op=ALU.max)
            sv = pd.tile([1, 1], F32, tag="sv", name="sv")
            nc.vector.tensor_scalar(sv[:], mn1[:], float(-DEPS), float(DEPS),
                                    op0=ALU.is_ge, op1=ALU.mult)
            sb_ps = pdps.tile([128, 1], F32, tag="sb_ps", name="sb_ps")
            nc.tensor.matmul(sb_ps[:], C["ones1x128"][:], sv[:],
                             start=True, stop=True)
            nc.scalar.copy(shiftb[:], sb_ps[:])
            d0 = pd.tile([128, 2048], F16, tag="d0", name="d0")
            nc.vector.tensor_scalar_add(d0[:], iband[:], shiftb[:])
            nc.sync.dma_start(dp3[:, :, G:G + 512], r3(d0))
            if G > 0:
                z = pd.tile([128, 4, G], F16, tag="zpad", name="zpad")
                nc.vector.memset(z[:], 0.0)
                nc.sync.dma_start(dp3[:, :, 0:G], z[:])
                nc.sync.dma_start(dp3[:, :, G + 512:G + 512 + G], z[:])

        # ============================== phase E: FFT blocks =================
        with tc.tile_pool(name="pe", bufs=3) as pe, \
             tc.tile_pool(name="pe_ps", bufs=4, space="PSUM") as peps:
            for (yi, xi) in blocks:
                y, x = YS[yi], XS[xi]
                h = YE[yi] - y
                w = XE[xi] - x
                w2 = w // 2 + 1
                blk = yi * 5 + xi
                Chm = C[f"chm{h}"][0:h, 0:h]
                Shm = C[f"shm{h}"][0:h, 0:h]
                Shn = C[f"shneg{h}"][0:h, 0:h]
                idh = C["id16"][0:h, 0:h]

                B = pe.tile([128, 128], F16, tag="B", name="B")
                nc.sync.dma_start(B[0:h, 0:w],
                                  depth_pad[y:y + h, G + x:G + x + w])
                bt_ps = peps.tile([128, 128], F16, tag="fps16", name="fps16")
                nc.tensor.transpose(bt_ps[0:w, 0:h], B[0:h, 0:w], idh)
                BT = pe.tile([128, 128], F16, tag="BT", name="BT")
                nc.scalar.copy(BT[0:w, 0:h], bt_ps[0:w, 0:h])

                xr_ps = peps.tile([128, 128], F32, tag="fps", name="fps")
                xi_ps = peps.tile([128, 128], F32, tag="fps", name="fps")
                nc.tensor.matmul(xr_ps[0:h, 0:w2], BT[0:w, 0:h],
                                 C[f"cwt{w}"][0:w, 0:w2], start=True, stop=True)
                nc.tensor.matmul(xi_ps[0:h, 0:w2], BT[0:w, 0:h],
                                 C[f"swt{w}"][0:w, 0:w2], start=True, stop=True)
                Xr = pe.tile([128, 65], F16, tag="Xr", name="Xr")
                Xi = pe.tile([128, 65], F16, tag="Xi", name="Xi")
                nc.scalar.copy(Xr[0:h, 0:w2], xr_ps[0:h, 0:w2])
                nc.vector.tensor_copy(Xi[0:h, 0:w2], xi_ps[0:h, 0:w2])

                yr_ps = peps.tile([128, 128], F32, tag="fps", name="fps")
                yi_ps = peps.tile([128, 128], F32, tag="fps", name="fps")
                nc.tensor.matmul(yr_ps[0:h, 0:w2], Chm, Xr[0:h, 0:w2],
                                 start=True, stop=False)
                nc.tensor.matmul(yr_ps[0:h, 0:w2], Shm, Xi[0:h, 0:w2],
                                 start=False, stop=True)
                nc.tensor.matmul(yi_ps[0:h, 0:w2], Chm, Xi[0:h, 0:w2],
                                 start=True, stop=False)
                nc.tensor.matmul(yi_ps[0:h, 0:w2], Shn, Xr[0:h, 0:w2],
                                 start=False, stop=True)

                outr = pe.tile([128, 65], F32, tag="outr", name="outr")
                nc.vector.tensor_scalar_mul(outr[0:h, 0:w2],
                                            C[f"kymat_{h}"][0:h, 0:w2],
                                            meansb[0:h, blk:blk + 1])
                nc.vector.scalar_tensor_tensor(
                    outr[0:h, 0:w2], C[f"kxmat_{w}"][0:h, 0:w2],
                    meansb[0:h, 25 + blk:26 + blk], outr[0:h, 0:w2],
                    op0=ALU.mult, op1=ALU.add)
                op1 = pe.tile([128, 65], F32, tag="op1", name="op1")
                nc.vector.tensor_scalar(op1[0:h, 0:w2], outr[0:h, 0:w2],
                                        float(-2.0 * L), 1.0,
                                        op0=ALU.mult, op1=ALU.add)
                op2 = pe.tile([128, 65], F32, tag="op2", name="op2")
                nc.vector.tensor_mul(op2[0:h, 0:w2], op1[0:h, 0:w2],
                                     op1[0:h, 0:w2])
                op8 = pe.tile([128, 65], F32, tag="op8", name="op8")
                nc.vector.tensor_mul(op8[0:h, 0:w2], op2[0:h, 0:w2],
                                     op2[0:h, 0:w2])
                nc.vector.tensor_mul(op8[0:h, 0:w2], op8[0:h, 0:w2],
                                     op8[0:h, 0:w2])
                nc.vector.tensor_mul(op8[0:h, 0:w2], op8[0:h, 0:w2],
                                     op2[0:h, 0:w2])  # ^10

                Zr = pe.tile([128, 65], F16, tag="Zr", name="Zr")
                Zi = pe.tile([128, 65], F16, tag="Zi", name="Zi")
                nc.vector.tensor_mul(Zr[0:h, 0:w2], yr_ps[0:h, 0:w2],
                                     op8[0:h, 0:w2])
                nc.vector.tensor_mul(Zi[0:h, 0:w2], yi_ps[0:h, 0:w2],
                                     op8[0:h, 0:w2])

                xpr_ps = peps.tile([128, 128], F32, tag="fps", name="fps")
                xpi_ps = peps.tile([128, 128], F32, tag="fps", name="fps")
                nc.tensor.matmul(xpr_ps[0:h, 0:w2], Chm, Zr[0:h, 0:w2],
                                 start=True, stop=False)
                nc.tensor.matmul(xpr_ps[0:h, 0:w2], Shn, Zi[0:h, 0:w2],
                                 start=False, stop=True)
                nc.tensor.matmul(xpi_ps[0:h, 0:w2], Chm, Zi[0:h, 0:w2],
                                 start=True, stop=False)
                nc.tensor.matmul(xpi_ps[0:h, 0:w2], Shm, Zr[0:h, 0:w2],
                                 start=False, stop=True)
                Xpr = pe.tile([128, 65], F16, tag="Xpr", name="Xpr")
                Xpi = pe.tile([128, 65], F16, tag="Xpi", name="Xpi")
                nc.scalar.copy(Xpr[0:h, 0:w2], xpr_ps[0:h, 0:w2])
                nc.vector.tensor_copy(Xpi[0:h, 0:w2], xpi_ps[0:h, 0:w2])

                xprt_ps = peps.tile([128, 128], F16, tag="fps16", name="fps16")
                xpit_ps = peps.tile([128, 128], F16, tag="fps16", name="fps16")
                nc.tensor.transpose(xprt_ps[0:w2, 0:h], Xpr[0:h, 0:w2], idh)
                nc.tensor.transpose(xpit_ps[0:w2, 0:h], Xpi[0:h, 0:w2], idh)
                XprT = pe.tile([65, 128], F16, tag="XprT", name="XprT")
                XpiT = pe.tile([65, 128], F16, tag="XpiT", name="XpiT")
                nc.scalar.copy(XprT[0:w2, 0:h], xprt_ps[0:w2, 0:h])
                nc.vector.tensor_copy(XpiT[0:w2, 0:h], xpit_ps[0:w2, 0:h])

                new_ps = peps.tile([128, 128], F32, tag="fps", name="fps")
                nc.tensor.matmul(new_ps[0:h, 0:w], XprT[0:w2, 0:h],
                                 C[f"amat_{h}_{w}"][0:w2, 0:w],
                                 start=True, stop=False)
                nc.tensor.matmul(new_ps[0:h, 0:w], XpiT[0:w2, 0:h],
                                 C[f"bmat_{h}_{w}"][0:w2, 0:w],
                                 start=False, stop=True)

                d = pe.tile([128, 128], F32, tag="d", name="d")
                nc.vector.tensor_sub(d[0:h, 0:w], new_ps[0:h, 0:w],
                                     B[0:h, 0:w])
                by_s = C["by128"][0:h, 0:1] if y > 0 else 1.0
                nc.vector.tensor_scalar(d[0:h, 0:w], d[0:h, 0:w],
                                        gateb[0:h, blk:blk + 1], by_s,
                                        op0=ALU.mult, op1=ALU.mult)
                if x > 0:
                    nc.vector.tensor_mul(d[0:h, 0:w], d[0:h, 0:w],
                                         C["bx128"][0:h, 0:w])
                upd = pe.tile([128, 128], F16, tag="upd", name="upd")
                nc.vector.tensor_add(upd[0:h, 0:w], B[0:h, 0:w], d[0:h, 0:w])
                nc.sync.dma_start(depth_pad[y:y + h, G + x:G + x + w],
                                  upd[0:h, 0:w])

        # ============================== phase F: scan =======================
        # fp16 datapath + persistent fp32 accumulator in PSUM:
        #   I_ps (PSUM, fp32) holds the running depth; each step
        #   scalar-engine casts it to fp16 (I16 / I16s shifted), DVE computes
        #   fluxes tv/th in fp16, and the tensor engine accumulates the
        #   shifted flux updates straight into I_ps via fp16 matmuls
        #   (id/negid/sdnneg @ 1 cyc/row vs 4 for fp32).
        sp = ctx.enter_context(tc.tile_pool(name="scan", bufs=1))
        ipsp = ctx.enter_context(
            tc.tile_pool(name="scan_ips", bufs=1, space="PSUM"))
        sps_up = ctx.enter_context(
            tc.tile_pool(name="scan_ps_up", bufs=2, space="PSUM"))
        dumps = ctx.enter_context(
            tc.tile_pool(name="scan_ps_dum", bufs=3, space="PSUM"))
        W3 = 3 * Wc
        I_ps = ipsp.tile([128, W4], F32, tag="I_ps", name="I_ps")
        I_ps3 = I_ps[:].rearrange("p (j w) -> p j w", j=4)
        I16 = sp.tile([128, W4], F16, tag="I16", name="I16")
        X = sp.tile([128, W4], F16, tag="X", name="X")
        dumsrc = sp.tile([128, 512], F16, tag="dumsrc", name="dumsrc")
        dv = sp.tile([128, W4], F16, tag="dv", name="dv")
        tv = sp.tile([128, W4], F16, tag="tv", name="tv")
        dhh = sp.tile([128, W4], F16, tag="dhh", name="dhh")
        th = sp.tile([128, W4], F16, tag="th", name="th")
        I0 = sp.tile([128, W4], F32, tag="I0", name="I0")
        Lcv = sp.tile([128, W4], F32, tag="Lcv", name="Lcv")
        Lch = sp.tile([128, W4], F32, tag="Lch", name="Lch")
        Lcv16 = sp.tile([128, W4], F16, tag="Lcv16", name="Lcv16")
        Lch16 = sp.tile([128, W4], F16, tag="Lch16", name="Lch16")
        nc.vector.memset(th[:], 0.0)
        nc.vector.memset(dumsrc[:], 0.25)

        def blend4(dst, shape3, mk_src, seli, dt=F32):
            """dst = sum_g mk_src(g) * seli[:, g];  tuple src => DMA from DRAM"""
            if nsplit == 1:
                s = mk_src(0)
                if isinstance(s, tuple):
                    nc.sync.dma_start(dst, s[0])
                else:
                    nc.vector.tensor_copy(dst, s)
                return
            srcs = []
            for g_ in range(nsplit):
                s = mk_src(g_)
                if isinstance(s, tuple):
                    stmp = sp.tile(shape3, dt, tag=f"b4t{g_}",
                                   name=f"b4t{g_}")
                    nc.sync.dma_start(stmp[:], s[0])
                    s = stmp[:]
                srcs.append(s)
            for g_, s in enumerate(srcs):
                if g_ == 0:
                    nc.vector.tensor_scalar_mul(dst, s, seli[:, 0:1])
                else:
                    nc.vector.scalar_tensor_tensor(
                        dst, s, seli[:, g_:g_ + 1], dst,
                        op0=ALU.mult, op1=ALU.add)

        I03 = I0[:].rearrange("p (j w) -> p j w", j=4)
        blend4(I03, [128, 4, Wc],
               lambda g_: (dp3[:, :, g_ * OWN:g_ * OWN + Wc],), C["sel"],
               dt=F16)
        Lcv3 = Lcv[:].rearrange("p (j w) -> p j w", j=4)
        Lch3 = Lch[:].rearrange("p (j w) -> p j w", j=4)
        blend4(Lcv3, [128, 4, Wc],
               lambda g_: cv[:, :, P0 - G + g_ * OWN:P0 - G + g_ * OWN + Wc],
               C["sel"])
        blend4(Lch3, [128, 4, Wc],
               lambda g_: ch[:, :, P0 - G + g_ * OWN:P0 - G + g_ * OWN + Wc],
               C["sel"])
        nc.scalar.mul(Lcv16[:], Lcv[:], float(L))
        nc.scalar.mul(Lch16[:], Lch[:], float(L))
        Lch163 = Lch16[:].rearrange("p (j w) -> p j w", j=4)
        nc.vector.memset(Lch163[:, :, Wc - 1], 0.0)

        def chunks(lo, hi):
            r = []
            while lo < hi:
                nx = min((lo // 512 + 1) * 512, hi)
                r.append((lo, nx))
                lo = nx
            return r

        # init I_ps = I0 via fp32 identity matmuls (start=True opens the
        # persistent accumulation group; it stays open across all steps)
        for (a, b) in chunks(0, W4):
            nc.tensor.matmul(I_ps[:, a:b], C["id128"][:], I0[:, a:b],
                             start=True, stop=False, skip_group_check=True)

        if nsplit > 1:
            cc_in = dram.tile([2, H, G], F16)
            cc_out = dram.tile([nsplit, 2, H, G], F16)
            cc3i = cc_in[:].rearrange("s (p j) g -> s p j g", p=128)
            cc3o = cc_out[:].rearrange("n s (p j) g -> n s p j g", p=128)
            edgeL = sp.tile([128, 4, G], F16, tag="edgeL", name="edgeL")
            edgeR = sp.tile([128, 4, G], F16, tag="edgeR", name="edgeR")
            halo = sp.tile([128, nsplit, 2, 4, G], F16, tag="halo",
                           name="halo")
            cc4o = cc_out[:].rearrange("n s (p j) g -> p n s j g", p=128)

        def acc(olo, ohi, lhs, r_t, roff=None, stop=False):
            """I_ps[:, k] += sum lhs-shifted r_t, chunked at PSUM banks"""
            if roff is None:
                roff = olo
            for (a, b) in chunks(olo, ohi):
                nc.tensor.matmul(
                    I_ps[:, a:b], lhs, r_t[:, a - olo + roff:b - olo + roff],
                    start=False, stop=stop, skip_group_check=True)

        steps_done = 0
        while steps_done < niter:
            seg = min(G, niter - steps_done) if nsplit > 1 else niter
            for s_ in range(seg):
                last = (steps_done + s_ + 1 == niter)
                nc.scalar.copy(I16[:, 0:Wc], I_ps[:, 0:Wc])
                nc.scalar.copy(I16[:, Wc:W4], I_ps[:, Wc:W4])
                up_ps = sps_up.tile([128, Wc], F32, tag="up_ps", name="up_ps")
                nc.tensor.matmul(up_ps[:], C["sup16"][:], I16[:, 0:Wc],
                                 start=True, stop=True, skip_group_check=True)
                nc.vector.tensor_sub(dv[:, 0:W3], I_ps[:, Wc:W4],
                                     I16[:, 0:W3])
                nc.vector.tensor_sub(dv[:, W3:W4], up_ps[:], I16[:, W3:W4])
                nc.vector.tensor_mul(tv[:], Lcv16[:], dv[:])
                nc.vector.tensor_sub(dhh[:, 0:W4 - 1], I_ps[:, 1:W4],
                                     I16[:, 0:W4 - 1])
                nc.vector.tensor_mul(th[:, 0:W4 - 1], Lch16[:, 0:W4 - 1],
                                     dhh[:, 0:W4 - 1])
                nc.vector.tensor_add(X[:], tv[:], th[:])
                # I[k] += tv[k] - tv[k-Wc] + th[k] - th[k-1]
                acc(Wc, W4, C["negid16"][:], tv, roff=0)
                acc(0, Wc, C["sdnneg16"][:], tv, roff=W3)
                acc(1, W4, C["negid16"][:], th, roff=0)
                acc(0, W4, C["id16"][:], X, stop=last)
                if not last:
                    for _ in range(ndum):
                        dps = dumps.tile([128, 512], F32, tag="dps",
                                         name="dps")
                        nc.tensor.matmul(dps[:], C["id16"][:],
                                         dumsrc[:, 0:512], start=True,
                                         stop=True, skip_group_check=True)
            steps_done += seg
            if nsplit > 1 and steps_done < niter:
                for _ in range(ndum_exch):
                    dps = dumps.tile([128, 512], F32, tag="dps", name="dps")
                    nc.tensor.matmul(dps[:], C["id16"][:], dumsrc[:, 0:512],
                                     start=True, stop=True,
                                     skip_group_check=True)
            if nsplit > 1 and steps_done < niter:
                nc.vector.tensor_copy(edgeL[:], I_ps3[:, :, G:2 * G])
                nc.scalar.copy(edgeR[:], I_ps3[:, :, Wc - 2 * G:Wc - G])
                nc.sync.dma_start(cc3i[0], edgeL[:])
                nc.sync.dma_start(cc3i[1], edgeR[:])
                nc.gpsimd.collective_compute(
                    "AllGather", ALU.bypass,
                    replica_groups=[[0, 1, 2, 3], [4, 5, 6, 7]],
                    ins=[cc_in.opt()], outs=[cc_out.opt()])
                nc.sync.dma_start(halo[:], cc4o)
                blend4(I_ps3[:, :, 0:G], [128, 4, G],
                       lambda g_: halo[:, g_, 1], C["selprev"], dt=F16)
                blend4(I_ps3[:, :, Wc - G:Wc], [128, 4, G],
                       lambda g_: halo[:, g_, 0], C["selnext"], dt=F16)

        o3 = out[:].rearrange("(p j) w -> p j w", p=128)
        otp = ctx.enter_context(tc.tile_pool(name="otp", bufs=4))
        for g_ in range(nsplit):
            ot = otp.tile([128, 4, OWN], F32, tag="ot", name="ot")
            nc.vector.tensor_scalar(
                ot[:], I_ps3[:, :, G:G + OWN], shiftb[:],
                C["sel"][:, g_:g_ + 1],
                op0=ALU.subtract, op1=ALU.mult)
            nc.sync.dma_start(o3[:, :, g_ * OWN:(g_ + 1) * OWN], ot[:])

    nc.compile()
    return nc


# ---------------------------------------------------------------- entry point
_CACHE = {}


def _get_program(nsplit=4, G=16, niter=NITER, ndum=6, ndum_exch=20):
    key = (nsplit, G, niter, ndum, ndum_exch)
    if key not in _CACHE:
        _CACHE[key] = build_program(nsplit=nsplit, G=G, niter=niter,
                                    ndum=ndum, ndum_exch=ndum_exch)
    return _CACHE[key]


def make_in_maps(guide, initial, nsplit=4):
    guide = np.ascontiguousarray(np.asarray(guide, dtype=np.float32))
    initial = np.ascontiguousarray(np.asarray(initial, dtype=np.float32))
    Bn = guide.shape[0]
    ncores = 8
    per_img = ncores // Bn
    consts = make_consts()
    in_maps = []
    for core in range(ncores):
        b = core // per_img
        g = (core % per_img) % nsplit
        m = dict(consts)
        m.update(make_core_consts(g, nsplit))
        m["guide"] = guide[b]
        m["initial"] = initial[b, 0]
        in_maps.append(m)
    return in_maps


def kernel(guide, initial, nsplit=4, G=16, niter=NITER, _return_raw=False,
           ndum=6, ndum_exch=20, **run_kwargs):
    guide = np.ascontiguousarray(np.asarray(guide, dtype=np.float32))
    initial = np.ascontiguousarray(np.asarray(initial, dtype=np.float32))
    Bn = guide.shape[0]
    assert guide.shape == (Bn, 3, H, W) and initial.shape == (Bn, 1, H, W)
    nc = _get_program(nsplit, G, niter, ndum, ndum_exch)
    in_maps = make_in_maps(guide, initial, nsplit)
    ncores = 8
    per_img = ncores // Bn
    res = run_bass_kernel_spmd(nc, in_maps, list(range(ncores)), **run_kwargs)
    outs = []
    for b in range(Bn):
        acc = np.zeros((H, W), np.float32)
        for g in range(min(nsplit, per_img)):
            acc += res.results[b * per_img + g]["out"]
        outs.append(acc)
    full = np.stack(outs)[:, None].astype(np.float32)
    if _return_raw:
        return full, res
    return full



# revision 13
# speedup vs baseline: 1.1484x; 1.1484x over previous
"""Trainium2 Bass kernel for nn_FFTDiffuseBase.

Self-contained: builds constants on host, emits a Tile/Bass SPMD program for
8 NeuronCores (2 images x 4-way column split with ghost-halo exchange),
runs via bass_utils.run_bass_kernel_spmd, gathers the full output.
"""
import numpy as np
from contextlib import ExitStack

import concourse.bass as bass
import concourse.tile as tile
from concourse import bacc
from concourse import mybir
from concourse.bass_utils import run_bass_kernel_spmd

F32 = mybir.dt.float32
F16 = mybir.dt.float16
ALU = mybir.AluOpType
ACTF = mybir.ActivationFunctionType

BS = 128
OVERLAP = 16
NITER = 250
L = 0.24
FFT_STEPS = 10
DEPS = 0.1
H = 512
W = 512

K = np.exp(np.log(np.float32(0.03), dtype=np.float32), dtype=np.float32)

YS = list(range(0, H, BS - OVERLAP))          # [0,112,224,336,448]
XS = list(range(0, W, BS - OVERLAP))
YE = [min(y + BS, H) for y in YS]
XE = [min(x + BS, W) for x in XS]


# ---------------------------------------------------------------- host consts
def dft_mats(h, w):
    w2 = w // 2 + 1
    n = np.arange(w)
    kw = np.arange(w2)
    ang_w = 2 * np.pi * np.outer(kw, n) / w
    CwT = np.cos(ang_w).T.astype(np.float32)
    SwT = (-np.sin(ang_w)).T.astype(np.float32)
    m = np.arange(h)
    ang_h = 2 * np.pi * np.outer(m, m) / h
    Ch = np.cos(ang_h).astype(np.float32)
    Sh = np.sin(ang_h).astype(np.float32)
    alpha = np.full(w2, 2.0)
    alpha[0] = 1.0
    if w % 2 == 0:
        alpha[-1] = 1.0
    A = (alpha[:, None] * np.cos(ang_w) / (h * w)).astype(np.float32)
    Bm = (-alpha[:, None] * np.sin(ang_w) / (h * w)).astype(np.float32)
    return CwT, SwT, Ch, Sh, A, Bm


_CONSTS_CACHE = {}


def make_consts():
    if "c" in _CONSTS_CACHE:
        return _CONSTS_CACHE["c"]
    c = {}
    ii = np.eye(128, dtype=np.float32)
    c["id128"] = ii
    c["negid128"] = -ii
    sup = np.zeros((128, 128), np.float32)   # out[m] = in[m+1]
    for m_ in range(127):
        sup[m_ + 1, m_] = 1.0
    c["sup128"] = sup
    sdn = np.zeros((128, 128), np.float32)   # out[m] = in[m-1]
    for m_ in range(1, 128):
        sdn[m_ - 1, m_] = 1.0
    c["sdn128"] = sdn
    c["sdnneg128"] = -sdn
    c["id16"] = ii.astype(np.float16)
    c["negid16"] = (-ii).astype(np.float16)
    c["sup16"] = sup.astype(np.float16)
    c["sdnneg16"] = (-sdn).astype(np.float16)
    for h in (128, 64):
        _, _, Ch, Sh, _, _ = dft_mats(h, h)
        c[f"chm{h}"] = Ch.astype(np.float16)
        c[f"shm{h}"] = Sh.astype(np.float16)
        c[f"shneg{h}"] = (-Sh).astype(np.float16)
        ky = np.arange(h).astype(np.float32) * np.float32(2 * np.pi / h)
        ky1c = (np.float32(1.0) - np.cos(ky)).astype(np.float32)
        c[f"kymat_{h}"] = np.tile(ky1c[:, None], (1, 65)).copy()       # [h,65]
    for w in (128, 64):
        w2 = w // 2 + 1
        CwT, SwT, _, _, _, _ = dft_mats(128, w)
        c[f"cwt{w}"] = CwT.astype(np.float16)
        c[f"swt{w}"] = SwT.astype(np.float16)
        kx = np.arange(w2).astype(np.float32) * np.float32(2 * np.pi / w)
        kx1c = (np.float32(1.0) - np.cos(kx)).astype(np.float32)
        c[f"kxmat_{w}"] = np.tile(kx1c[None, :], (128, 1)).copy()      # [128,w2]
    for h in (128, 64):
        for w in (128, 64):
            _, _, _, _, A, Bm = dft_mats(h, w)
            c[f"amat_{h}_{w}"] = A.astype(np.float16)
            c[f"bmat_{h}_{w}"] = Bm.astype(np.float16)
    bx = np.where(np.arange(128) < OVERLAP, np.arange(128) / OVERLAP, 1.0)
    c["bx128"] = np.tile(bx.astype(np.float32)[None, :], (128, 1)).copy()
    c["by128"] = bx.astype(np.float32)[:, None].copy()                 # [128,1]
    bmfull = np.zeros((128, 5), np.float32)
    bmcorrneg = np.zeros((128, 5), np.float32)
    for yr in range(5):
        bmfull[YS[yr] // 4: YE[yr] // 4, yr] = 1.0
        if YE[yr] < H:
            bmcorrneg[YE[yr] // 4 - 1, yr] = -1.0
    c["bmfull"] = bmfull
    c["bmcorrneg"] = bmcorrneg
    selyr = np.zeros((5, 5 * 128), np.float32)
    for yr in range(5):
        selyr[yr, yr * 128:(yr + 1) * 128] = 1.0
    c["selyr"] = selyr
    for q in ("cv", "ch", "ur"):
        cnt = np.zeros((5, 5), np.float32)
        for yr in range(5):
            ye = {"cv": min(YE[yr] - 1, H - 1), "ch": YE[yr],
                  "ur": min(YE[yr], H - 1)}[q]
            for xr in range(5):
                xe = {"cv": min(XE[xr], W), "ch": min(XE[xr] - 1, W - 1),
                      "ur": min(XE[xr], W - 1)}[q]
                cnt[yr, xr] = 1.0 / ((ye - YS[yr]) * (xe - XS[xr]))
        c[f"cnt_{q}"] = cnt
    c["ones1x128"] = np.ones((1, 128), np.float32)
    _CONSTS_CACHE["c"] = c
    return c


def make_core_consts(g, nsplit):
    sel = np.zeros((128, nsplit), np.float32)
    sel[:, g] = 1.0
    selprev = np.zeros((128, nsplit), np.float32)
    if g > 0:
        selprev[:, g - 1] = 1.0
    selnext = np.zeros((128, nsplit), np.float32)
    if g < nsplit - 1:
        selnext[:, g + 1] = 1.0
    return {"sel": sel, "selprev": selprev, "selnext": selnext}


def r3(t, j=4):
    """[128, (j w)] AP -> [128, j, w]"""
    return t[:].rearrange("p (j w) -> p j w", j=j)


# ---------------------------------------------------------------- program
def build_program(nsplit=4, G=16, niter=NITER, nblocks=None,
                  ndum=8, ndum_exch=20):
    assert 512 % nsplit == 0
    OWN = 512 // nsplit
    if nsplit == 1:
        G = 0
    Wc = OWN + 2 * G
    W4 = 4 * Wc
    P0 = max(G, 1)
    Wp2 = 512 + 2 * P0
    Wp = 512 + 2 * G

    nc = bacc.Bacc("TRN2", target_bir_lowering=False, num_devices=8)

    guide = nc.dram_tensor("guide", [3, H, W], F32, kind="ExternalInput")
    initial = nc.dram_tensor("initial", [H, W], F32, kind="ExternalInput")
    out = nc.dram_tensor("out", [H, W], F32, kind="ExternalOutput")

    consts = make_consts()

    def _dt(v):
        if v.dtype == np.float32:
            return F32
        if v.dtype == np.float16:
            return F16
        return mybir.dt.bfloat16
    cdram = {k: nc.dram_tensor(k, list(v.shape), _dt(v), kind="ExternalInput")
             for k, v in consts.items()}
    for k in ("sel", "selprev", "selnext"):
        cdram[k] = nc.dram_tensor(k, [128, nsplit], F32, kind="ExternalInput")

    blocks = [(yi, xi) for yi in range(5) for xi in range(5)]
    blocks.sort(key=lambda b: (2 * b[0] + b[1], b[0]))   # wavefront order
    if nblocks is not None:
        blocks = blocks[:nblocks]

    with tile.TileContext(nc) as tc, ExitStack() as ctx:
        cp = ctx.enter_context(tc.tile_pool(name="consts", bufs=1))
        C = {}
        for k, dh_ in cdram.items():
            t = cp.tile(list(dh_.shape), dh_.dtype, tag=k)
            nc.sync.dma_start(t[:], dh_[:])
            C[k] = t

        dram = ctx.enter_context(tc.tile_pool(name="dram", bufs=1, space="DRAM"))
        depth_pad = dram.tile([H, Wp], F16)

        coef = ctx.enter_context(tc.tile_pool(name="coef", bufs=1))
        cv = coef.tile([128, 4, Wp2], F32, tag="cv", name="cv")
        ch = coef.tile([128, 4, Wp2], F32, tag="ch", name="ch")
        urf = coef.tile([128, 4, 512], F32, tag="urf", name="urf")
        meansb = coef.tile([128, 75], F32, tag="meansb", name="meansb")
        gateb = coef.tile([128, 25], F32, tag="gateb", name="gateb")
        shiftb = coef.tile([128, 1], F32, tag="shiftb", name="shiftb")
        zrow = coef.tile([1, 512], F32, tag="zrow", name="zrow")
        nc.vector.memset(zrow[:], 0.0)

        # ============================== phase A: cv/ch from guide ===========
        with tc.tile_pool(name="pa", bufs=1) as pa, \
             tc.tile_pool(name="pa_ps", bufs=3, space="PSUM") as paps:
            nc.vector.memset(cv[:], 0.0)
            nc.gpsimd.memset(ch[:], 0.0)
            accv = pa.tile([128, 2048], F32, tag="accv", name="accv")
            acch = pa.tile([128, 2048], F32, tag="acch", name="acch")
            nc.gpsimd.memset(acch[:], 0.0)
            gb = [pa.tile([128, 2048], F32, tag=f"gb{c_}", name=f"gb{c_}") for c_ in range(3)]
            for c_ in range(3):
                nc.sync.dma_start(
                    gb[c_][:],
                    guide[c_].rearrange("(p j) w -> p (j w)", p=128))
            for c_ in range(3):
                fr = paps.tile([128, 512], F32, tag="fr", name="fr")
                nc.tensor.matmul(fr[:], C["sup128"][:], gb[c_][:, 0:512],
                                 start=True, stop=True)
                tmp = pa.tile([128, 2048], F32, tag="tmpv", name="tmpv")
                nc.vector.tensor_sub(tmp[:, 0:1536], gb[c_][:, 512:2048],
                                     gb[c_][:, 0:1536])
                nc.vector.tensor_sub(tmp[:, 1536:2048], fr[:],
                                     gb[c_][:, 1536:2048])
                if c_ == 0:
                    nc.vector.scalar_tensor_tensor(
                        accv[:], tmp[:], -1.0, tmp[:],
                        op0=ALU.mult, op1=ALU.max)
                else:
                    ab = pa.tile([128, 2048], F32, tag="absv", name="absv")
                    nc.vector.scalar_tensor_tensor(
                        ab[:], tmp[:], -1.0, tmp[:], op0=ALU.mult, op1=ALU.max)
                    nc.vector.tensor_add(accv[:], accv[:], ab[:])
                tmph = pa.tile([128, 2048], F32, tag="tmph", name="tmph")
                nc.gpsimd.tensor_sub(tmph[:, 0:2047], gb[c_][:, 1:2048],
                                     gb[c_][:, 0:2047])
                if c_ == 0:
                    nc.vector.scalar_tensor_tensor(
                        acch[:, 0:2047], tmph[:, 0:2047], -1.0,
                        tmph[:, 0:2047], op0=ALU.mult, op1=ALU.max)
                else:
                    ab2 = pa.tile([128, 2048], F32, tag="absh", name="absh")
                    nc.vector.scalar_tensor_tensor(
                        ab2[:, 0:2047], tmph[:, 0:2047], -1.0,
                        tmph[:, 0:2047], op0=ALU.mult, op1=ALU.max)
                    nc.gpsimd.tensor_add(acch[:, 0:2047], acch[:, 0:2047],
                                         ab2[:, 0:2047])
            s13k = float(1.0 / (3.0 * float(K)))
            for acc, dst in ((accv, cv), (acch, ch)):
                t1 = pa.tile([128, 2048], F32, tag="gt1", name="gt1")
                nc.scalar.activation(t1[:], acc[:], ACTF.Square, scale=s13k)
                nc.vector.tensor_scalar_add(t1[:], t1[:], 1.0)
                nc.vector.reciprocal_approx_fast(dst[:, :, P0:P0 + 512],
                                                 r3(t1))
            nc.sync.dma_start(cv[127:128, 3, P0:P0 + 512],
                              zrow[0:1, 0:512])                  # row 511
            nc.vector.memset(ch[:, :, P0 + 511], 0.0)            # col 511

        # ============================== phase B: ur =========================
        with tc.tile_pool(name="pb", bufs=1) as pb, \
             tc.tile_pool(name="pb_ps", bufs=4, space="PSUM") as pbps:
            varr = {}
            for side, src in (("cv", cv), ("ch", ch)):
                sq = pb.tile([128, 4, Wp2], F32, tag="sq", name="sq")
                nc.vector.memset(sq[:], 0.0)
                nc.scalar.activation(sq[:, :, P0:P0 + 512],
                                     src[:, :, P0:P0 + 512], ACTF.Square)
                for which, xt in (("s1", src), ("s2", sq)):
                    hs = pb.tile([128, 4, 512], F32, tag=f"hs{which}", name=f"hs{which}")
                    a0 = xt[:, :, P0 - 1:P0 - 1 + 512]
                    a1 = xt[:, :, P0:P0 + 512]
                    a2 = xt[:, :, P0 + 1:P0 + 1 + 512]
                    nc.vector.tensor_add(hs[:], a0, a1)
                    nc.vector.tensor_add(hs[:], hs[:], a2)
                    hsf = hs[:].rearrange("p j w -> p (j w)")
                    vs = pb.tile([128, 2048], F32, tag=f"vs{which}", name=f"vs{which}")
                    nc.vector.tensor_add(vs[:, 512:1536], hsf[:, 0:1024],
                                         hsf[:, 512:1536])
                    nc.vector.tensor_add(vs[:, 512:1536], vs[:, 512:1536],
                                         hsf[:, 1024:2048])
                    pv = pbps.tile([128, 512], F32, tag="pv", name="pv")
                    nc.tensor.matmul(pv[:], C["sdn128"][:], hsf[:, 1536:2048],
                                     start=True, stop=True)
                    nv = pbps.tile([128, 512], F32, tag="nv", name="nv")
                    nc.tensor.matmul(nv[:], C["sup128"][:], hsf[:, 0:512],
                                     start=True, stop=True)
                    nc.vector.tensor_add(vs[:, 0:512], hsf[:, 0:512],
                                         hsf[:, 512:1024])
                    nc.vector.tensor_add(vs[:, 0:512], vs[:, 0:512], pv[:])
                    nc.vector.tensor_add(vs[:, 1536:2048], hsf[:, 1024:1536],
                                         hsf[:, 1536:2048])
                    nc.vector.tensor_add(vs[:, 1536:2048], vs[:, 1536:2048],
                                         nv[:])
                    varr[which] = vs
                vv = pb.tile([128, 2048], F32, tag=f"vv{side}", name=f"vv{side}")
                nc.vector.tensor_scalar_mul(vv[:], varr["s2"][:],
                                            float(1.0 / 9.0))
                m1 = pb.tile([128, 2048], F32, tag="m1", name="m1")
                nc.scalar.activation(m1[:], varr["s1"][:], ACTF.Square,
                                     scale=float(1.0 / 9.0))
                nc.vector.tensor_sub(vv[:], vv[:], m1[:])
                varr[side] = vv
            ua = pb.tile([128, 2048], F32, tag="ua", name="ua")
            nc.vector.tensor_scalar(ua[:], varr["cv"][:], 0.1, None,
                                    op0=ALU.is_lt)
            nc.vector.scalar_tensor_tensor(
                urf[:].rearrange("p j w -> p (j w)"), varr["ch"][:], 0.1,
                ua[:], op0=ALU.is_lt, op1=ALU.mult)
            nc.sync.dma_start(urf[127:128, 3, :], zrow[0:1, 0:512])  # row 511

        # ============================== phase C: block means ================
        with tc.tile_pool(name="pc", bufs=1) as pc, \
             tc.tile_pool(name="pc_ps", bufs=1, space="PSUM") as pcps:
            sums = {}
            for q, src, corr in (
                    ("cv", cv[:, :, P0:P0 + 512], True),
                    ("ch", ch[:, :, P0:P0 + 512], False),
                    ("ur", urf[:, :, :], False)):
                rg = pcps.tile([128, 4, 512], F32, tag="rg", name="rg")
                for jj in range(4):
                    nc.tensor.matmul(rg[0:5, jj, :], C["bmfull"][:],
                                     src[:, jj, :], start=True,
                                     stop=(jj < 3 or not corr))
                if corr:
                    nc.tensor.matmul(rg[0:5, 3, :], C["bmcorrneg"][:],
                                     src[:, 3, :], start=False, stop=True)
                sm = pc.tile([5, 5], F32, tag=f"sm{q}", name=f"sm{q}")
                for xr in range(5):
                    x0 = XS[xr]
                    x1 = {"cv": min(XE[xr], W), "ch": min(XE[xr] - 1, W - 1),
                          "ur": min(XE[xr], W - 1)}[q]
                    nc.vector.tensor_reduce(
                        sm[0:5, xr:xr + 1], rg[0:5, :, x0:x1],
                        axis=mybir.AxisListType.XY, op=ALU.add)
                nc.vector.tensor_mul(sm[:], sm[:], C[f"cnt_{q}"][:])
                sums[q] = sm
            mscr = dram.tile([75], F32, name="mscr")
            mscr3 = mscr[:].rearrange("(q a b) -> q a b", a=5, b=5)
            for qi, q in enumerate(("cv", "ch", "ur")):
                nc.sync.dma_start(mscr3[qi], sums[q][:])
            nc.sync.dma_start(meansb[:], mscr[:].partition_broadcast(128))
            nc.vector.tensor_scalar(gateb[:], meansb[:, 50:75], 0.7, None,
                                    op0=ALU.is_gt)

        # ============================== phase D: shift + depth init =========
        dp3 = depth_pad[:].rearrange("(p j) w -> p j w", p=128)
        with tc.tile_pool(name="pd", bufs=1) as pd, \
             tc.tile_pool(name="pd_ps", bufs=1, space="PSUM") as pdps:
            iband = pd.tile([128, 2048], F32, tag="iband", name="iband")
            nc.sync.dma_start(iband[:],
                              initial[:].rearrange("(p j) w -> p (j w)", p=128))
            mn = pd.tile([128, 1], F32, tag="mn", name="mn")
            nc.vector.tensor_reduce(mn[:], iband[:],
                                    axis=mybir.AxisListType.X, op=ALU.min)
            nc.vector.tensor_scalar_mul(mn[:], mn[:], -1.0)
            mn1 = pd.tile([1, 1], F32, tag="mn1", name="mn1")
            nc.gpsimd.tensor_reduce(mn1[:], mn[:],
                                    axis=mybir.AxisListType.C, op=ALU.max)
            sv = pd.tile([1, 1], F32, tag="sv", name="sv")
            nc.vector.tensor_scalar(sv[:], mn1[:], float(-DEPS), float(DEPS),
                                    op0=ALU.is_ge, op1=ALU.mult)
            sb_ps = pdps.tile([128, 1], F32, tag="sb_ps", name="sb_ps")
            nc.tensor.matmul(sb_ps[:], C["ones1x128"][:], sv[:],
                             start=True, stop=True)
            nc.scalar.copy(shiftb[:], sb_ps[:])
            d0 = pd.tile([128, 2048], F16, tag="d0", name="d0")
            nc.vector.tensor_scalar_add(d0[:], iband[:], shiftb[:])
            nc.sync.dma_start(dp3[:, :, G:G + 512], r3(d0))
            if G > 0:
                z = pd.tile([128, 4, G], F16, tag="zpad", name="zpad")
                nc.vector.memset(z[:], 0.0)
                nc.sync.dma_start(dp3[:, :, 0:G], z[:])
                nc.sync.dma_start(dp3[:, :, G + 512:G + 512 + G], z[:])

        # ============================== phase E: FFT blocks =================
        with tc.tile_pool(name="pe", bufs=3) as pe, \
             tc.tile_pool(name="pe_ps", bufs=4, space="PSUM") as peps:
            for (yi, xi) in blocks:
                y, x = YS[yi], XS[xi]
                h = YE[yi] - y
                w = XE[xi] - x
                w2 = w // 2 + 1
                blk = yi * 5 + xi
                Chm = C[f"chm{h}"][0:h, 0:h]
                Shm = C[f"shm{h}"][0:h, 0:h]
                Shn = C[f"shneg{h}"][0:h, 0:h]
                idh = C["id16"][0:h, 0:h]

                B = pe.tile([128, 128], F16, tag="B", name="B")
                nc.sync.dma_start(B[0:h, 0:w],
                                  depth_pad[y:y + h, G + x:G + x + w])
                bt_ps = peps.tile([128, 128], F16, tag="fps16", name="fps16")
                nc.tensor.transpose(bt_ps[0:w, 0:h], B[0:h, 0:w], idh)
                BT = pe.tile([128, 128], F16, tag="BT", name="BT")
                nc.scalar.copy(BT[0:w, 0:h], bt_ps[0:w, 0:h])

                xr_ps = peps.tile([128, 128], F32, tag="fps", name="fps")
                xi_ps = peps.tile([128, 128], F32, tag="fps", name="fps")
                nc.tensor.matmul(xr_ps[0:h, 0:w2], BT[0:w, 0:h],
                                 C[f"cwt{w}"][0:w, 0:w2], start=True, stop=True)
                nc.tensor.matmul(xi_ps[0:h, 0:w2], BT[0:w, 0:h],
                                 C[f"swt{w}"][0:w, 0:w2], start=True, stop=True)
                Xr = pe.tile([128, 65], F16, tag="Xr", name="Xr")
                Xi = pe.tile([128, 65], F16, tag="Xi", name="Xi")
                nc.scalar.copy(Xr[0:h, 0:w2], xr_ps[0:h, 0:w2])
                nc.vector.tensor_copy(Xi[0:h, 0:w2], xi_ps[0:h, 0:w2])

                yr_ps = peps.tile([128, 128], F32, tag="fps", name="fps")
                yi_ps = peps.tile([128, 128], F32, tag="fps", name="fps")
                nc.tensor.matmul(yr_ps[0:h, 0:w2], Chm, Xr[0:h, 0:w2],
                                 start=True, stop=False)
                nc.tensor.matmul(yr_ps[0:h, 0:w2], Shm, Xi[0:h, 0:w2],
                                 start=False, stop=True)
                nc.tensor.matmul(yi_ps[0:h, 0:w2], Chm, Xi[0:h, 0:w2],
                                 start=True, stop=False)
                nc.tensor.matmul(yi_ps[0:h, 0:w2], Shn, Xr[0:h, 0:w2],
                                 start=False, stop=True)

                outr = pe.tile([128, 65], F32, tag="outr", name="outr")
                nc.vector.tensor_scalar_mul(outr[0:h, 0:w2],
                                            C[f"kymat_{h}"][0:h, 0:w2],
                                            meansb[0:h, blk:blk + 1])
                nc.vector.scalar_tensor_tensor(
                    outr[0:h, 0:w2], C[f"kxmat_{w}"][0:h, 0:w2],
                    meansb[0:h, 25 + blk:26 + blk], outr[0:h, 0:w2],
                    op0=ALU.mult, op1=ALU.add)
                op1 = pe.tile([128, 65], F32, tag="op1", name="op1")
                nc.vector.tensor_scalar(op1[0:h, 0:w2], outr[0:h, 0:w2],
                                        float(-2.0 * L), 1.0,
                                        op0=ALU.mult, op1=ALU.add)
                op2 = pe.tile([128, 65], F32, tag="op2", name="op2")
                nc.vector.tensor_mul(op2[0:h, 0:w2], op1[0:h, 0:w2],
                                     op1[0:h, 0:w2])
                op8 = pe.tile([128, 65], F32, tag="op8", name="op8")
                nc.vector.tensor_mul(op8[0:h, 0:w2], op2[0:h, 0:w2],
                                     op2[0:h, 0:w2])
                nc.vector.tensor_mul(op8[0:h, 0:w2], op8[0:h, 0:w2],
                                     op8[0:h, 0:w2])
                nc.vector.tensor_mul(op8[0:h, 0:w2], op8[0:h, 0:w2],
                                     op2[0:h, 0:w2])  # ^10

                Zr = pe.tile([128, 65], F16, tag="Zr", name="Zr")
                Zi = pe.tile([128, 65], F16, tag="Zi", name="Zi")
                nc.vector.tensor_mul(Zr[0:h, 0:w2], yr_ps[0:h, 0:w2],
                                     op8[0:h, 0:w2])
                nc.vector.tensor_mul(Zi[0:h, 0:w2], yi_ps[0:h, 0:w2],
                                     op8[0:h, 0:w2])

                xpr_ps = peps.tile([128, 128], F32, tag="fps", name="fps")
                xpi_ps = peps.tile([128, 128], F32, tag="fps", name="fps")
                nc.tensor.matmul(xpr_ps[0:h, 0:w2], Chm, Zr[0:h, 0:w2],
                                 start=True, stop=False)
                nc.tensor.matmul(xpr_ps[0:h, 0:w2], Shn, Zi[0:h, 0:w2],
                                 start=False, stop=True)
                nc.tensor.matmul(xpi_ps[0:h, 0:w2], Chm, Zi[0:h, 0:w2],
                                 start=True, stop=False)
                nc.tensor.matmul(xpi_ps[0:h, 0:w2], Shm, Zr[0:h, 0:w2],
                                 start=False, stop=True)
                Xpr = pe.tile([128, 65], F16, tag="Xpr", name="Xpr")
                Xpi = pe.tile([128, 65], F16, tag="Xpi", name="Xpi")
                nc.scalar.copy(Xpr[0:h, 0:w2], xpr_ps[0:h, 0:w2])
                nc.vector.tensor_copy(Xpi[0:h, 0:w2], xpi_ps[0:h, 0:w2])

                xprt_ps = peps.tile([128, 128], F16, tag="fps16", name="fps16")
                xpit_ps = peps.tile([128, 128], F16, tag="fps16", name="fps16")
                nc.tensor.transpose(xprt_ps[0:w2, 0:h], Xpr[0:h, 0:w2], idh)
                nc.tensor.transpose(xpit_ps[0:w2, 0:h], Xpi[0:h, 0:w2], idh)
                XprT = pe.tile([65, 128], F16, tag="XprT", name="XprT")
                XpiT = pe.tile([65, 128], F16, tag="XpiT", name="XpiT")
                nc.scalar.copy(XprT[0:w2, 0:h], xprt_ps[0:w2, 0:h])
                nc.vector.tensor_copy(XpiT[0:w2, 0:h], xpit_ps[0:w2, 0:h])

                new_ps = peps.tile([128, 128], F32, tag="fps", name="fps")
                nc.tensor.matmul(new_ps[0:h, 0:w], XprT[0:w2, 0:h],
                                 C[f"amat_{h}_{w}"][0:w2, 0:w],
                                 start=True, stop=False)
                nc.tensor.matmul(new_ps[0:h, 0:w], XpiT[0:w2, 0:h],
                                 C[f"bmat_{h}_{w}"][0:w2, 0:w],
                                 start=False, stop=True)

                d = pe.tile([128, 128], F32, tag="d", name="d")
                nc.vector.tensor_sub(d[0:h, 0:w], new_ps[0:h, 0:w],
                                     B[0:h, 0:w])
                by_s = C["by128"][0:h, 0:1] if y > 0 else 1.0
                nc.vector.tensor_scalar(d[0:h, 0:w], d[0:h, 0:w],
                                        gateb[0:h, blk:blk + 1], by_s,
                                        op0=ALU.mult, op1=ALU.mult)
                if x > 0:
                    nc.vector.tensor_mul(d[0:h, 0:w], d[0:h, 0:w],
                                         C["bx128"][0:h, 0:w])
                upd = pe.tile([128, 128], F16, tag="upd", name="upd")
                nc.vector.tensor_add(upd[0:h, 0:w], B[0:h, 0:w], d[0:h, 0:w])
                nc.sync.dma_start(depth_pad[y:y + h, G + x:G + x + w],
                                  upd[0:h, 0:w])

        # ============================== phase F: scan =======================
        # fp16 datapath + persistent fp32 accumulator in PSUM:
        #   I_ps (PSUM, fp32) holds the running depth; each step
        #   scalar-engine casts it to fp16 (I16 / I16s shifted), DVE computes
        #   fluxes tv/th in fp16, and the tensor engine accumulates the
        #   shifted flux updates straight into I_ps via fp16 matmuls
        #   (id/negid/sdnneg @ 1 cyc/row vs 4 for fp32).
        sp = ctx.enter_context(tc.tile_pool(name="scan", bufs=1))
        ipsp = ctx.enter_context(
            tc.tile_pool(name="scan_ips", bufs=1, space="PSUM"))
        sps_up = ctx.enter_context(
            tc.tile_pool(name="scan_ps_up", bufs=2, space="PSUM"))
        dumps = ctx.enter_context(
            tc.tile_pool(name="scan_ps_dum", bufs=3, space="PSUM"))
        W3 = 3 * Wc
        I_ps = ipsp.tile([128, W4], F32, tag="I_ps", name="I_ps")
        I_ps3 = I_ps[:].rearrange("p (j w) -> p j w", j=4)
        I16 = sp.tile([128, W4], F16, tag="I16", name="I16")
        X = sp.tile([128, W4], F16, tag="X", name="X")
        dumsrc = sp.tile([128, 512], F16, tag="dumsrc", name="dumsrc")
        dv = sp.tile([128, W4], F16, tag="dv", name="dv")
        tv = sp.tile([128, W4], F16, tag="tv", name="tv")
        dhh = sp.tile([128, W4], F16, tag="dhh", name="dhh")
        th = sp.tile([128, W4], F16, tag="th", name="th")
        I0 = sp.tile([128, W4], F32, tag="I0", name="I0")
        Lcv = sp.tile([128, W4], F32, tag="Lcv", name="Lcv")
        Lch = sp.tile([128, W4], F32, tag="Lch", name="Lch")
        Lcv16 = sp.tile([128, W4], F16, tag="Lcv16", name="Lcv16")
        Lch16 = sp.tile([128, W4], F16, tag="Lch16", name="Lch16")
        nc.vector.memset(th[:], 0.0)
        nc.vector.memset(dumsrc[:], 0.25)

        def blend4(dst, shape3, mk_src, seli, dt=F32):
            """dst = sum_g mk_src(g) * seli[:, g];  tuple src => DMA from DRAM"""
            if nsplit == 1:
                s = mk_src(0)
                if isinstance(s, tuple):
                    nc.sync.dma_start(dst, s[0])
                else:
                    nc.vector.tensor_copy(dst, s)
                return
            srcs = []
            for g_ in range(nsplit):
                s = mk_src(g_)
                if isinstance(s, tuple):
                    stmp = sp.tile(shape3, dt, tag=f"b4t{g_}",
                                   name=f"b4t{g_}")
                    nc.sync.dma_start(stmp[:], s[0])
                    s = stmp[:]
                srcs.append(s)
            for g_, s in enumerate(srcs):
                if g_ == 0:
                    nc.vector.tensor_scalar_mul(dst, s, seli[:, 0:1])
                else:
                    nc.vector.scalar_tensor_tensor(
                        dst, s, seli[:, g_:g_ + 1], dst,
                        op0=ALU.mult, op1=ALU.add)

        I03 = I0[:].rearrange("p (j w) -> p j w", j=4)
        blend4(I03, [128, 4, Wc],
               lambda g_: (dp3[:, :, g_ * OWN:g_ * OWN + Wc],), C["sel"],
               dt=F16)
        Lcv3 = Lcv[:].rearrange("p (j w) -> p j w", j=4)
        Lch3 = Lch[:].rearrange("p (j w) -> p j w", j=4)
        blend4(Lcv3, [128, 4, Wc],
               lambda g_: cv[:, :, P0 - G + g_ * OWN:P0 - G + g_ * OWN + Wc],
               C["sel"])
        blend4(Lch3, [128, 4, Wc],
               lambda g_: ch[:, :, P0 - G + g_ * OWN:P0 - G + g_ * OWN + Wc],
               C["sel"])
        nc.scalar.mul(Lcv16[:], Lcv[:], float(L))
        nc.scalar.mul(Lch16[:], Lch[:], float(L))
        Lch163 = Lch16[:].rearrange("p (j w) -> p j w", j=4)
        nc.vector.memset(Lch163[:, :, Wc - 1], 0.0)

        def chunks(lo, hi):
            r = []
            while lo < hi:
                nx = min((lo // 512 + 1) * 512, hi)
                r.append((lo, nx))
                lo = nx
            return r

        # init I_ps = I0 via fp32 identity matmuls (start=True opens the
        # persistent accumulation group; it stays open across all steps)
        for (a, b) in chunks(0, W4):
            nc.tensor.matmul(I_ps[:, a:b], C["id128"][:], I0[:, a:b],
                             start=True, stop=False, skip_group_check=True)

        if nsplit > 1:
            cc_in = dram.tile([2, H, G], F16)
            cc_out = dram.tile([nsplit, 2, H, G], F16)
            cc3i = cc_in[:].rearrange("s (p j) g -> s p j g", p=128)
            cc3o = cc_out[:].rearrange("n s (p j) g -> n s p j g", p=128)
            edgeL = sp.tile([128, 4, G], F16, tag="edgeL", name="edgeL")
            edgeR = sp.tile([128, 4, G], F16, tag="edgeR", name="edgeR")
            halo = sp.tile([128, nsplit, 2, 4, G], F16, tag="halo",
                           name="halo")
            cc4o = cc_out[:].rearrange("n s (p j) g -> p n s j g", p=128)

        def acc(olo, ohi, lhs, r_t, roff=None, stop=False):
            """I_ps[:, k] += sum lhs-shifted r_t, chunked at PSUM banks"""
            if roff is None:
                roff = olo
            for (a, b) in chunks(olo, ohi):
                nc.tensor.matmul(
                    I_ps[:, a:b], lhs, r_t[:, a - olo + roff:b - olo + roff],
                    start=False, stop=stop, skip_group_check=True)

        steps_done = 0
        while steps_done < niter:
            seg = min(G, niter - steps_done) if nsplit > 1 else niter
            for s_ in range(seg):
                last = (steps_done + s_ + 1 == niter)
                nc.scalar.copy(I16[:, 0:Wc], I_ps[:, 0:Wc])
                nc.scalar.copy(I16[:, Wc:W4], I_ps[:, Wc:W4])
                up_ps = sps_up.tile([128, Wc], F32, tag="up_ps", name="up_ps")
                nc.tensor.matmul(up_ps[:], C["sup16"][:], I16[:, 0:Wc],
                                 start=True, stop=True, skip_group_check=True)
                nc.vector.tensor_sub(dv[:, 0:W3], I_ps[:, Wc:W4],
                                     I16[:, 0:W3])
                nc.vector.tensor_sub(dv[:, W3:W4], up_ps[:], I16[:, W3:W4])
                nc.vector.tensor_mul(tv[:], Lcv16[:], dv[:])
                nc.vector.tensor_sub(dhh[:, 0:W4 - 1], I_ps[:, 1:W4],
                                     I16[:, 0:W4 - 1])
                nc.vector.tensor_mul(th[:, 0:W4 - 1], Lch16[:, 0:W4 - 1],
                                     dhh[:, 0:W4 - 1])
                nc.vector.tensor_add(X[:], tv[:], th[:])
                # I[k] += tv[k] - tv[k-Wc] + th[k] - th[k-1]
                acc(Wc, W4, C["negid16"][:], tv, roff=0)
                acc(0, Wc, C["sdnneg16"][:], tv, roff=W3)
                acc(1, W4, C["negid16"][:], th, roff=0)
                acc(0, W4, C["id16"][:], X, stop=last)
                if not last:
                    for _ in range(ndum):
                        dps = dumps.tile([128, 512], F32, tag="dps",
                                         name="dps")
                        nc.tensor.matmul(dps[:], C["id16"][:],
                                         dumsrc[:, 0:512], start=True,
                                         stop=True, skip_group_check=True)
            steps_done += seg
            if nsplit > 1 and steps_done < niter:
                for _ in range(ndum_exch):
                    dps = dumps.tile([128, 512], F32, tag="dps", name="dps")
                    nc.tensor.matmul(dps[:], C["id16"][:], dumsrc[:, 0:512],
                                     start=True, stop=True,
                                     skip_group_check=True)
            if nsplit > 1 and steps_done < niter:
                nc.vector.tensor_copy(edgeL[:], I_ps3[:, :, G:2 * G])
                nc.scalar.copy(edgeR[:], I_ps3[:, :, Wc - 2 * G:Wc - G])
                nc.sync.dma_start(cc3i[0], edgeL[:])
                nc.sync.dma_start(cc3i[1], edgeR[:])
                nc.gpsimd.collective_compute(
                    "AllGather", ALU.bypass,
                    replica_groups=[[0, 1, 2, 3], [4, 5, 6, 7]],
                    ins=[cc_in.opt()], outs=[cc_out.opt()])
                nc.sync.dma_start(halo[:], cc4o)
                blend4(I_ps3[:, :, 0:G], [128, 4, G],
                       lambda g_: halo[:, g_, 1], C["selprev"], dt=F16)
                blend4(I_ps3[:, :, Wc - G:Wc], [128, 4, G],
                       lambda g_: halo[:, g_, 0], C["selnext"], dt=F16)

        o3 = out[:].rearrange("(p j) w -> p j w", p=128)
        otp = ctx.enter_context(tc.tile_pool(name="otp", bufs=4))
        for g_ in range(nsplit):
            ot = otp.tile([128, 4, OWN], F32, tag="ot", name="ot")
            nc.vector.tensor_scalar(
                ot[:], I_ps3[:, :, G:G + OWN], shiftb[:],
                C["sel"][:, g_:g_ + 1],
                op0=ALU.subtract, op1=ALU.mult)
            nc.sync.dma_start(o3[:, :, g_ * OWN:(g_ + 1) * OWN], ot[:])

    nc.compile()
    return nc


# ---------------------------------------------------------------- entry point
_CACHE = {}


def _get_program(nsplit=4, G=16, niter=NITER, ndum=8, ndum_exch=20):
    key = (nsplit, G, niter, ndum, ndum_exch)
    if key not in _CACHE:
        _CACHE[key] = build_program(nsplit=nsplit, G=G, niter=niter,
                                    ndum=ndum, ndum_exch=ndum_exch)
    return _CACHE[key]


def make_in_maps(guide, initial, nsplit=4):
    guide = np.ascontiguousarray(np.asarray(guide, dtype=np.float32))
    initial = np.ascontiguousarray(np.asarray(initial, dtype=np.float32))
    Bn = guide.shape[0]
    ncores = 8
    per_img = ncores // Bn
    consts = make_consts()
    in_maps = []
    for core in range(ncores):
        b = core // per_img
        g = (core % per_img) % nsplit
        m = dict(consts)
        m.update(make_core_consts(g, nsplit))
        m["guide"] = guide[b]
        m["initial"] = initial[b, 0]
        in_maps.append(m)
    return in_maps


def kernel(guide, initial, nsplit=4, G=16, niter=NITER, _return_raw=False,
           ndum=8, ndum_exch=20, **run_kwargs):
    guide = np.ascontiguousarray(np.asarray(guide, dtype=np.float32))
    initial = np.ascontiguousarray(np.asarray(initial, dtype=np.float32))
    Bn = guide.shape[0]
    assert guide.shape == (Bn, 3, H, W) and initial.shape == (Bn, 1, H, W)
    nc = _get_program(nsplit, G, niter, ndum, ndum_exch)
    in_maps = make_in_maps(guide, initial, nsplit)
    ncores = 8
    per_img = ncores // Bn
    res = run_bass_kernel_spmd(nc, in_maps, list(range(ncores)), **run_kwargs)
    outs = []
    for b in range(Bn):
        acc = np.zeros((H, W), np.float32)
        for g in range(min(nsplit, per_img)):
            acc += res.results[b * per_img + g]["out"]
        outs.append(acc)
    full = np.stack(outs)[:, None].astype(np.float32)
    if _return_raw:
        return full, res
    return full

